# revision 18
# baseline (speedup 1.0000x reference)
"""Trainium2 Bass kernel for a 2-layer LSTM decoder + vocab projection + log-softmax.

Parallelization (8 cores, symmetric SPMD):
  - Each core owns a 1024-row slice of the 8192 LSTM gate rows of BOTH layers
    (256 rows from each of the i,f,o,g blocks -> core c owns h-dims
    [256c, 256c+256)), and a 4000-col slice of the vocab projection.
  - Per tick: matmul h @ W_hh.T slice -> LSTM cell elementwise -> AllGather of
    the 256-dim h slice. Two collectives per tick: AG_A carries hn0(t)
    (critical path), AG_B carries hn1(j) and is consumed a full tick later so
    its latency is hidden. Layer 1 lags LAG=3 ticks; its input matmul batches
    D=2 steps per weight pass.
  - Projection: out[128 tokens x 1000 vocab] quarter-tiles; q0/q1 of each
    group are interleaved into the recurrence (weights resident, activations
    straight from the SBUF h1 ring), q2/q3 run in a tensor-bound tail.
  - Vocab-sharded log-softmax: per-quarter max/sum stats, 2 tiny AllReduces
    for the global normalizer. Device outputs bf16 logits + f32 logZ; the
    final (logits - logZ) subtraction happens on host (same rounding as
    doing it on-device from bf16 logits).

Numerics: bf16 weights/h/logits with fp32 accumulation.
"""

import numpy as np
import ml_dtypes
from contextlib import ExitStack

import concourse.bass as bass
import concourse.mybir as mybir
import concourse.tile as tile
from concourse import bacc
from concourse import bass_utils

F32 = mybir.dt.float32
BF16 = mybir.dt.bfloat16
I32 = mybir.dt.int32
AF = mybir.ActivationFunctionType
bf16 = ml_dtypes.bfloat16

H = 1024
RH = 2048
V = 32000
B = 32
T = 128
NC = 8
GS = 1024          # gate rows per core per layer
HS = 256           # h dims per core
VS = V // NC       # vocab cols per core
VQ = VS // 4       # vocab quarter
D = 2              # layer-1 input-matmul batching (steps per weight pass)
LAG = D + 1        # layer-1 step lag behind layer 0
H0S = 8            # h0 ring slots
H1S = 8            # h1 ring slots
SOS_ID = 1

TS_FULL = T - 1    # 127 recurrence steps


def _p_major(w, kt, mt):
    """(kt*128, mt*128) -> (128, kt*mt*128) packed [p, k*mt*128 + m*128 + q]."""
    return np.ascontiguousarray(
        w.reshape(kt, 128, mt, 128).transpose(1, 0, 2, 3).reshape(128, kt * mt * 128)
    )


def _ktile_cols(a):
    """(kt*128, n) -> (128, kt*n) packed [p, k*n + j] = a[128k+p, j]."""
    kt = a.shape[0] // 128
    return np.ascontiguousarray(
        a.reshape(kt, 128, a.shape[1]).transpose(1, 0, 2).reshape(128, kt * a.shape[1])
    )


def prep_inputs(inp, ts=TS_FULL):
    """Host-side prep: slice/transpose/cast weights per core -> in_maps."""
    ntok_pad = ((ts * B + 127) // 128) * 128
    f32 = np.float32

    emb = np.asarray(inp["emb"], f32).astype(bf16)
    tb = np.asarray(inp["target_batch"]).astype(np.int64)
    idx = tb[:, :ts].T.reshape(-1).astype(np.int32)       # (ts*B,) t-major
    idx = np.concatenate([idx, np.zeros(ntok_pad - idx.size, np.int32)])
    idx = np.ascontiguousarray(idx.reshape(ntok_pad // 128, 128).T)  # [p, group]

    ch = np.asarray(inp["context_h"], f32)
    cc = np.asarray(inp["context_c"], f32)
    h_init = np.concatenate([ch[0::2], ch[1::2]], axis=2)  # (2, B, RH)
    c_init = np.concatenate([cc[0::2], cc[1::2]], axis=2)

    def h_pack(hl):  # (B, RH) -> (128, 512) bf16 [p, 32k+b] = h[b, 128k+p]
        return np.ascontiguousarray(
            hl.T.reshape(16, 128, B).transpose(1, 0, 2).reshape(128, 16 * B)
        ).astype(bf16)

    ident = np.eye(128, dtype=bf16)

    Wih = [np.asarray(inp["W_ih0"], f32), np.asarray(inp["W_ih1"], f32)]
    Whh = [np.asarray(inp["W_hh0"], f32), np.asarray(inp["W_hh1"], f32)]
    bsum = [np.asarray(inp["b_ih0"], f32) + np.asarray(inp["b_hh0"], f32),
            np.asarray(inp["b_ih1"], f32) + np.asarray(inp["b_hh1"], f32)]
    W_out = np.asarray(inp["W_out"], f32)
    b_out = np.asarray(inp["b_out"], f32)

    in_maps = []
    for c in range(NC):
        # gate rows for core c, in i,f,o,g chunk order (256 rows each)
        rows = np.concatenate([np.arange(RH * k + HS * c, RH * k + HS * (c + 1))
                               for k in (0, 1, 3, 2)])  # i,f,o,g
        wih0t = _p_major(Wih[0][rows].T.astype(bf16), 8, 8)       # (128, 8192)
        whh0t = _p_major(Whh[0][rows].T.astype(bf16), 16, 8)      # (128, 16384)
        wih1t = _p_major(Wih[1][rows].T.astype(bf16), 16, 8)
        whh1t = _p_major(Whh[1][rows].T.astype(bf16), 16, 8)
        b0 = np.ascontiguousarray(bsum[0][rows].reshape(8, 128).T)  # (128, 8)
        b1 = np.ascontiguousarray(bsum[1][rows].reshape(8, 128).T)
        b1row = np.ascontiguousarray(bsum[1][rows].reshape(1, 1024)).astype(bf16)
        woutt = _ktile_cols(W_out[VS * c:VS * (c + 1)].T.astype(bf16))  # (128, 64000)
        boutc = b_out[VS * c:VS * (c + 1)].reshape(1, VS).astype(bf16)

        def c_pack(cl):  # (B, RH) slice -> (128, 64) f32
            s = cl[:, HS * c:HS * (c + 1)].T  # (256, B)
            return np.ascontiguousarray(
                s.reshape(2, 128, B).transpose(1, 0, 2).reshape(128, 2 * B))

        in_maps.append({
            "idx": idx, "embt": emb, "ident": ident,
            "wih0t": wih0t, "whh0t": whh0t, "wih1t": wih1t, "whh1t": whh1t,
            "b0": b0, "b1": b1, "b1row": b1row, "woutt": woutt, "boutc": boutc,
            "h0init": h_pack(h_init[0]), "h1init": h_pack(h_init[1]),
            "c0init": c_pack(c_init[0]), "c1init": c_pack(c_init[1]),
        })
    return in_maps, ntok_pad


def build_nc(ts=TS_FULL):
    ntok_pad = ((ts * B + 127) // 128) * 128
    ntok = ts * B
    ngrp = ntok_pad // 128          # token groups of 128 for the projection
    nticks = ts + LAG + 1

    nc = bacc.Bacc("TRN2", target_bir_lowering=False, debug=False,
                   enable_asserts=False, num_devices=NC)

    # ---- I/O ----
    idx_t = nc.dram_tensor("idx", [128, ntok_pad // 128], I32,
                           kind="ExternalInput").ap()
    emb_t = nc.dram_tensor("embt", [V, H], BF16, kind="ExternalInput").ap()
    ident_t = nc.dram_tensor("ident", [128, 128], BF16, kind="ExternalInput").ap()
    wih0_t = nc.dram_tensor("wih0t", [128, 8 * GS], BF16, kind="ExternalInput").ap()
    whh0_t = nc.dram_tensor("whh0t", [128, 16 * GS], BF16, kind="ExternalInput").ap()
    wih1_t = nc.dram_tensor("wih1t", [128, 16 * GS], BF16, kind="ExternalInput").ap()
    whh1_t = nc.dram_tensor("whh1t", [128, 16 * GS], BF16, kind="ExternalInput").ap()
    b0_t = nc.dram_tensor("b0", [128, 8], F32, kind="ExternalInput").ap()
    b1_t = nc.dram_tensor("b1", [128, 8], F32, kind="ExternalInput").ap()
    b1r_t = nc.dram_tensor("b1row", [1, 1024], BF16, kind="ExternalInput").ap()
    wout_t = nc.dram_tensor("woutt", [128, 16 * VS], BF16, kind="ExternalInput").ap()
    bout_t = nc.dram_tensor("boutc", [1, VS], BF16, kind="ExternalInput").ap()
    h0i_t = nc.dram_tensor("h0init", [128, 512], BF16, kind="ExternalInput").ap()
    h1i_t = nc.dram_tensor("h1init", [128, 512], BF16, kind="ExternalInput").ap()
    c0i_t = nc.dram_tensor("c0init", [128, 64], F32, kind="ExternalInput").ap()
    c1i_t = nc.dram_tensor("c1init", [128, 64], F32, kind="ExternalInput").ap()
    logit_t = nc.dram_tensor("logits", [ntok_pad, VS], BF16,
                             kind="ExternalOutput").ap()
    logz_t = nc.dram_tensor("logz", [128, ngrp], F32, kind="ExternalOutput").ap()

    RG = [list(range(NC))]

    with ExitStack() as ctx:
        tc = ctx.enter_context(tile.TileContext(nc))
        dram = ctx.enter_context(tc.tile_pool(name="dram", bufs=1, space="DRAM"))
        agp = ctx.enter_context(tc.tile_pool(name="agp", bufs=6, space="DRAM"))
        keep = ctx.enter_context(tc.tile_pool(name="keep", bufs=1))

        # stats + projection constants live across phase 1 and phase 2
        m4 = keep.tile([128, 4 * ngrp], F32, tag="m4")
        s4 = keep.tile([128, 4 * ngrp], F32, tag="s4")
        ones_s = keep.tile([1, 128], BF16, tag="ones")
        bout_s = keep.tile([1, VS], BF16, tag="bouts")

        # persistent DRAM
        xbf_d = dram.tile([ntok_pad, H], BF16, tag="xbf")
        g0_d = dram.tile([8, 128, ntok], BF16, tag="g0d")
        outs_d = dram.tile([ngrp, 16, 128, 128], BF16, tag="outsd")

        with tc.tile_pool(name="rp", bufs=1) as rp:
            # ---- recurrence-lifetime SBUF ----
            whh0_s = rp.tile([128, 16 * GS], BF16, tag="whh0s")
            b0_s = rp.tile([128, 8], F32, tag="b0s")
            b1_s = rp.tile([128, 8], F32, tag="b1s")
            b1r_s = rp.tile([1, 1024], BF16, tag="b1rs")
            ident_s = rp.tile([128, 128], BF16, tag="idents")
            # h0 ring: [p, slot, k, b]; h1 ring: [p, k, slot, b] (k-major so a
            # projection lhsT [p, 4 steps x 32 b] slice is contiguous)
            h0ring = rp.tile([128, H0S * 512], BF16, tag="h0ring")
            h1ring = rp.tile([128, 16 * H1S * B], BF16, tag="h1ring")
            g0ring = rp.tile([128, 2 * 2048], BF16, tag="g0ring")   # 2 blocks x 8 steps
            g1ring = rp.tile([128, 8 * D * B], BF16, tag="g1ring")  # [p, m, s, b]
            h0i_s = rp.tile([128, 512], BF16, tag="h0is")
            h1i_s = rp.tile([128, 512], BF16, tag="h1is")
            nc.gpsimd.memset(ones_s[:], 1.0)
            nc.sync.dma_start(bout_s[:], bout_t[:])

            nc.sync.dma_start(whh0_s[:], whh0_t[:])
            nc.sync.dma_start(b0_s[:], b0_t[:])
            nc.sync.dma_start(b1_s[:], b1_t[:])
            nc.sync.dma_start(b1r_s[:], b1r_t[:])
            nc.sync.dma_start(ident_s[:], ident_t[:])
            nc.sync.dma_start(h0i_s[:], h0i_t[:])
            nc.sync.dma_start(h1i_s[:], h1i_t[:])

            # ============ Phase 0: embeddings + G0 = X @ Wih0.T + b0 ============
            TH = 2048  # token half for XT chunking
            with tc.tile_pool(name="p0sb", bufs=2) as p0sb, \
                 tc.tile_pool(name="p0ev", bufs=2) as p0ev, \
                 tc.tile_pool(name="p0big", bufs=1) as p0big, \
                 tc.tile_pool(name="p0ps", bufs=2, space="PSUM") as p0ps:
                idxs = p0big.tile([128, ntok_pad // 128], I32, tag="idxs")
                nc.sync.dma_start(idxs[:], idx_t[:])
                for it in range(ntok_pad // 128):
                    xg = p0sb.tile([128, H], BF16, tag="xg")
                    nc.gpsimd.indirect_dma_start(
                        out=xg[:], out_offset=None, in_=emb_t[:],
                        in_offset=bass.IndirectOffsetOnAxis(
                            ap=idxs[:, it:it + 1], axis=0))
                    nc.scalar.dma_start(xbf_d[128 * it:128 * (it + 1), :], xg[:])

                wih0_s = p0big.tile([128, 8 * GS], BF16, tag="wih0s")
                nc.sync.dma_start(wih0_s[:], wih0_t[:])
                xt_s = p0big.tile([128, 8 * TH], BF16, tag="xts")

                for half in range((ntok + TH - 1) // TH):
                    t0 = TH * half
                    tw = min(TH, ntok - t0)
                    twp = ((tw + 15) // 16) * 16  # transpose src rows mult of 16
                    for k in range(8):
                        nc.sync.dma_start_transpose(
                            xt_s[:, TH * k:TH * k + twp],
                            xbf_d[t0:t0 + twp, 128 * k:128 * (k + 1)])
                    nch = [(512 * i, min(512, tw - 512 * i))
                           for i in range((tw + 511) // 512)]
                    for m in range(8):
                        ps = p0ps.tile([128, 2048], F32, tag="p0ps")
                        for k in range(8):
                            lhs = wih0_s[:, k * GS + 128 * m: k * GS + 128 * (m + 1)]
                            for (o, w) in nch:
                                nc.tensor.matmul(
                                    ps[:, o:o + w], lhs,
                                    xt_s[:, TH * k + o: TH * k + o + w],
                                    start=(k == 0), stop=(k == 7))
                        ev = p0ev.tile([128, TH], BF16, tag="g0ev")
                        nc.scalar.activation(ev[:, :tw], ps[:, :tw], AF.Identity,
                                             bias=b0_s[:, m:m + 1])
                        nc.scalar.dma_start(g0_d[m, :, t0:t0 + tw], ev[:, :tw])

            # ============ Phase 1: recurrence ============
            c_prev = [None, None]
            with tc.tile_pool(name="rp2", bufs=1) as rp2, \
                 tc.tile_pool(name="ps0p", bufs=2, space="PSUM") as ps0_pool, \
                 tc.tile_pool(name="ps1p", bufs=2, space="PSUM") as ps1_pool, \
                 tc.tile_pool(name="psg1", bufs=2, space="PSUM") as psg1_pool, \
                 tc.tile_pool(name="psq", bufs=1, space="PSUM") as psq_pool, \
                 tc.tile_pool(name="pscr", bufs=1) as pscr_pool, \
                 tc.tile_pool(name="cell", bufs=2) as cell_pool:

                # layer-1 weights: first needed at tick LAG, loaded after phase 0
                wih1_s = rp2.tile([128, 16 * GS], BF16, tag="wih1s")
                whh1_s = rp2.tile([128, 16 * GS], BF16, tag="whh1s")
                nc.scalar.dma_start(wih1_s[:], wih1_t[:])
                nc.scalar.dma_start(whh1_s[:], whh1_t[:])
                # resident first TWO vocab-quarters of W_out: q0/q1 of every
                # group run interleaved into the ticks
                wout_qa = rp2.tile([128, 16 * 2 * VQ], BF16, tag="woutqa")
                nc.scalar.dma_start(
                    wout_qa[:].rearrange("p (k v) -> p k v", k=16),
                    wout_t[:].rearrange("p (k v) -> p k v", k=16)[:, :, 0:2 * VQ])
                nchq = [(0, 512), (512, VQ - 512)]

                h0r4 = h0ring[:].rearrange("p (s k b) -> p s k b", s=H0S, b=B)
                h1r4 = h1ring[:].rearrange("p (k s b) -> p k s b", s=H1S, b=B)
                h0i4 = h0i_s[:].rearrange("p (k b) -> p k b", b=B)
                h1i4 = h1i_s[:].rearrange("p (k b) -> p k b", b=B)
                g0r5 = g0ring[:].rearrange("p (h m s b) -> p h m s b",
                                           h=2, m=8, b=B)
                g1r4 = g1ring[:].rearrange("p (m s b) -> p m s b", m=8, b=B)

                proj_ps = [None]

                def proj_half(g, q, half):
                    # half a [128 tok x VQ vocab] projection tile (k 0-7 or
                    # 8-15); the second half adds bias + softmax stats.
                    # Split so each piece fits in one tick's AllGather shadow
                    # without delaying the next tick's critical hh matmul.
                    gh = 4 * g + q
                    v0 = VQ * q
                    s0 = (4 * g) % H1S
                    if half == 0:
                        psq = psq_pool.tile([128, 1024], F32, tag="psq")
                        proj_ps[0] = psq
                    psq = proj_ps[0]
                    for k in range(8 * half, 8 * half + 8):
                        lhs = h1r4[:, k, s0:s0 + 4, :]
                        for (o, w) in nchq:
                            nc.tensor.matmul(psq[:, o:o + w], lhs,
                                             wout_qa[:, 2 * VQ * k + v0 + o:
                                                     2 * VQ * k + v0 + o + w],
                                             start=(k == 0), stop=False)
                    if half == 0:
                        return
                    for (o, w) in nchq:
                        nc.tensor.matmul(psq[:, o:o + w], ones_s[:, :],
                                         bout_s[:, v0 + o:v0 + o + w],
                                         start=False, stop=True)
                    lsb = pscr_pool.tile([128, VQ], BF16, tag="lsb")
                    nc.vector.tensor_copy(lsb[:], psq[:, :VQ])
                    nc.scalar.dma_start(
                        logit_t[128 * g:128 * (g + 1), v0:v0 + VQ], lsb[:])

                def g0_prefetch(blk):
                    """DMA G0 steps [8*blk, 8*blk+8) -> g0ring half blk%2."""
                    t0 = 8 * blk
                    nsteps = min(8, ts - t0)
                    if nsteps <= 0:
                        return
                    dst = g0ring[:].rearrange("p (h m s b) -> p h m s b",
                                              h=2, m=8, b=B)
                    src = g0_d[:, :, B * t0: B * (t0 + nsteps)].rearrange(
                        "m p sb -> p m sb")
                    nc.scalar.dma_start(
                        dst[:, blk % 2, :, 0:nsteps, :].rearrange(
                            "p m s b -> p m (s b)"), src)

                def hh_matmul(w_s, rhs_of_k, ps, gadd_rhs):
                    # PSUM preload of the input-gate tensor via identity matmul
                    nc.tensor.matmul(ps[:, 0:256], ident_s[:], gadd_rhs,
                                     start=True, stop=False)
                    for m in range(8):
                        for k in range(16):
                            nc.tensor.matmul(
                                ps[:, B * m:B * (m + 1)],
                                w_s[:, k * GS + 128 * m: k * GS + 128 * (m + 1)],
                                rhs_of_k(k),
                                start=False, stop=(k == 15))

                def cell(l, ps):
                    """LSTM cell for layer l reading gates from PSUM directly."""
                    sfo = cell_pool.tile([128, 192], F32, tag=f"sfo{l}")
                    nc.scalar.activation(sfo[:], ps[:, 0:192], AF.Sigmoid)
                    tg = cell_pool.tile([128, 64], F32, tag=f"tg{l}")
                    nc.scalar.activation(tg[:], ps[:, 192:256], AF.Tanh)
                    t2 = cell_pool.tile([128, 64], F32, tag=f"t2{l}")
                    nc.vector.tensor_mul(t2[:], sfo[:, 64:128], c_prev[l][:])
                    t1 = cell_pool.tile([128, 64], F32, tag=f"t1{l}")
                    nc.vector.tensor_mul(t1[:], sfo[:, 0:64], tg[:])
                    cn = cell_pool.tile([128, 64], F32, tag=f"cn{l}")
                    nc.vector.tensor_add(cn[:], t1[:], t2[:])
                    c_prev[l] = cn
                    tcn = cell_pool.tile([128, 64], F32, tag=f"tc{l}")
                    nc.scalar.activation(tcn[:], cn[:], AF.Tanh)
                    hn = cell_pool.tile([128, 64], BF16, tag=f"hn{l}")
                    nc.vector.tensor_mul(hn[:], sfo[:, 128:192], tcn[:])
                    return hn

                c0s = cell_pool.tile([128, 64], F32, tag="cn0")
                nc.sync.dma_start(c0s[:], c0i_t[:])
                c_prev[0] = c0s
                c1s = cell_pool.tile([128, 64], F32, tag="cn1")
                nc.sync.dma_start(c1s[:], c1i_t[:])
                c_prev[1] = c1s

                g0_prefetch(0)
                g0_prefetch(1)
                done_quarters = set()
                pending = []
                proj_state = [None]

                agout_b_prev = None
                for t in range(nticks):
                    j = t - LAG  # layer-1 step this tick

                    # ---- layer 0, step t ----
                    if t < ts:
                        ps0 = ps0_pool.tile([128, 256], F32, tag="ps0")
                        rhs = (lambda k: h0i4[:, k, :]) if t == 0 else \
                            (lambda k, _s=(t - 1) % H0S: h0r4[:, _s, k, :])
                        hh_matmul(whh0_s, rhs, ps0,
                                  g0r5[:, (t // 8) % 2, :, t % 8, :])
                        if t % 8 == 7:
                            g0_prefetch(t // 8 + 2)
                        hn0 = cell(0, ps0)
                        aga_in = agp.tile([128, 64], BF16, tag="again")
                        # gpsimd queue: never blocked behind ring writes
                        nc.gpsimd.dma_start(aga_in[:], hn0[:])
                    else:
                        aga_in = agp.tile([128, 64], BF16, tag="again")

                    # h1 of step jj landed via last tick's AG_B; issued on
                    # the sync queue BEFORE the h0 ring write so its short
                    # AG_B wait neither blocks scalar compute nor queues
                    # behind this tick's AG_A
                    jj = t - 1 - LAG
                    if 0 <= jj < ts:
                        for jf in range(2):
                            nc.sync.dma_start(
                                h1r4[:, jf::2, jj % H1S, :],
                                agout_b_prev[:, :, B * jf:B * (jf + 1)]
                                .rearrange("r p b -> p r b"))
                        if jj % 4 == 3:
                            pending.append((jj // 4, 0))
                            pending.append((jj // 4, 1))

                    # critical collective: hn0(t) only
                    agout_a = agp.tile([NC, 128, 64], BF16, tag="agouta",
                                       addr_space="Shared")
                    nc.gpsimd.collective_compute(
                        "AllGather", mybir.AluOpType.bypass, replica_groups=RG,
                        ins=[aga_in[:].opt()], outs=[agout_a[:].opt()])
                    if t < ts:
                        nc.sync.dma_start(
                            h0r4[:, t % H0S, :, :].rearrange(
                                "p (r j) b -> p r (j b)", j=2),
                            agout_a[:].rearrange("r p jb -> p r jb"))

                    if 0 <= jj < ts:
                        # store h1 (= outs[jj]) for the phase-2 tail quarters
                        nc.sync.dma_start(
                            outs_d[jj // 4, :, :, B * (jj % 4):B * (jj % 4 + 1)]
                            .rearrange("k p b -> p k b"),
                            h1r4[:, :, jj % H1S, :])

                    # ---- layer 1, step j ----
                    if 0 <= j < ts:
                        if j % D == 0:
                            nb = min(D, ts - j)
                            psg = psg1_pool.tile([128, 8 * D * B], F32,
                                                 tag="psg1")
                            s0 = j % H0S
                            for m in range(8):
                                for k in range(16):
                                    nc.tensor.matmul(
                                        psg[:, D * B * m: D * B * m + B * nb],
                                        wih1_s[:, k * GS + 128 * m:
                                               k * GS + 128 * (m + 1)],
                                        h0r4[:, s0:s0 + nb, k, :],
                                        start=(k == 0), stop=False)
                                # bias add via K=1 matmul keeps the slow
                                # Identity evictions off the scalar queue
                                nc.tensor.matmul(
                                    psg[:, D * B * m: D * B * m + B * nb],
                                    b1r_s[:, 128 * m:128 * (m + 1)],
                                    ones_s[:, :B * nb],
                                    start=False, stop=True)
                            for m in range(8):
                                nc.vector.tensor_copy(
                                    g1ring[:, D * B * m: D * B * m + B * nb],
                                    psg[:, D * B * m: D * B * m + B * nb])

                        ps1 = ps1_pool.tile([128, 256], F32, tag="ps1t")
                        rhs = (lambda k: h1i4[:, k, :]) if j == 0 else \
                            (lambda k, _s=(j - 1) % H1S: h1r4[:, k, _s, :])
                        hh_matmul(whh1_s, rhs, ps1, g1r4[:, :, j % D, :])
                        hn1 = cell(1, ps1)
                        agb_in = agp.tile([128, 64], BF16, tag="agbin")
                        nc.gpsimd.dma_start(agb_in[:], hn1[:])
                    else:
                        agb_in = agp.tile([128, 64], BF16, tag="agbin")

                    # lagged collective: hn1(j), consumed next tick
                    agout_b = agp.tile([NC, 128, 64], BF16, tag="agoutb",
                                       addr_space="Shared")
                    nc.gpsimd.collective_compute(
                        "AllGather", mybir.AluOpType.bypass, replica_groups=RG,
                        ins=[agb_in[:].opt()], outs=[agout_b[:].opt()])
                    agout_b_prev = agout_b

                    # one projection half-quarter per tick, issued after the
                    # lagged collective so it fills the AllGather shadows
                    # without delaying hn1's gather
                    if proj_state[0] is not None:
                        g, q = proj_state[0]
                        proj_half(g, q, 1)
                        done_quarters.add((g, q))
                        proj_state[0] = None
                    elif pending:
                        proj_state[0] = pending.pop(0)
                        proj_half(proj_state[0][0], proj_state[0][1], 0)

                # drain any half-open quarter so its PSUM group is closed
                if proj_state[0] is not None:
                    g, q = proj_state[0]
                    proj_half(g, q, 1)
                    done_quarters.add((g, q))
                    proj_state[0] = None

        # ============ Phase 2 tail: remaining projection quarters ============
        with tc.tile_pool(name="p2keep", bufs=1) as p2keep:
            m_all = p2keep.tile([128, ngrp], F32, tag="mall")
            s_all = p2keep.tile([128, ngrp], F32, tag="sall")
            logz = p2keep.tile([128, ngrp], F32, tag="logz")

            with tc.tile_pool(name="p2w", bufs=1) as p2w, \
                 tc.tile_pool(name="p2sb", bufs=3) as p2sb, \
                 tc.tile_pool(name="p2scr", bufs=2) as p2scr, \
                 tc.tile_pool(name="p2ps", bufs=2, space="PSUM") as p2ps:
                wout_s = p2w.tile([128, 16 * VS], BF16, tag="wouts")
                nc.scalar.dma_start(wout_s[:], wout_t[:])

                nchq = [(0, 512), (512, VQ - 512)]
                for g in range(ngrp):
                    # stats for quarters computed during phase 1 (bf16 logits
                    # re-read from DRAM; hides under the PE-bound quarters)
                    for q in range(4):
                        if (g, q) not in done_quarters:
                            continue
                        gh = 4 * g + q
                        v0 = VQ * q
                        lgt = p2scr.tile([128, VQ], BF16, tag="lgt")
                        nc.sync.dma_start(
                            lgt[:], logit_t[128 * g:128 * (g + 1), v0:v0 + VQ])
                        nc.vector.tensor_reduce(m4[:, gh:gh + 1], lgt[:],
                                                axis=mybir.AxisListType.X,
                                                op=mybir.AluOpType.max)
                        negm = p2sb.tile([128, 1], F32, tag="negm")
                        nc.vector.tensor_scalar_mul(negm[:], m4[:, gh:gh + 1],
                                                    -1.0)
                        esc = p2scr.tile([128, VQ], BF16, tag="esc")
                        nc.scalar.activation(esc[:], lgt[:], AF.Exp,
                                             bias=negm[:, :1],
                                             accum_out=s4[:, gh:gh + 1])
                    rem = [q for q in range(4) if (g, q) not in done_quarters]
                    if not rem:
                        continue
                    osb = p2sb.tile([128, 2048], BF16, tag="osb")
                    nc.sync.dma_start(
                        osb[:].rearrange("p (k q) -> p k q", k=16),
                        outs_d[g, :, :, :].rearrange("k p q -> p k q"))
                    for q in rem:
                        v0 = VQ * q
                        gh = 4 * g + q
                        ps = p2ps.tile([128, 1024], F32, tag="p2ps")
                        for k in range(16):
                            lhs = osb[:, 128 * k:128 * (k + 1)]
                            for (o, w) in nchq:
                                nc.tensor.matmul(
                                    ps[:, o:o + w], lhs,
                                    wout_s[:, VS * k + v0 + o: VS * k + v0 + o + w],
                                    start=(k == 0), stop=False)
                        for (o, w) in nchq:
                            nc.tensor.matmul(ps[:, o:o + w], ones_s[:, :],
                                             bout_s[:, v0 + o: v0 + o + w],
                                             start=False, stop=True)
                        nc.vector.tensor_reduce(m4[:, gh:gh + 1], ps[:, :VQ],
                                                axis=mybir.AxisListType.X,
                                                op=mybir.AluOpType.max)
                        negm = p2sb.tile([128, 1], F32, tag="negm")
                        nc.vector.tensor_scalar_mul(negm[:], m4[:, gh:gh + 1], -1.0)
                        esc = p2scr.tile([128, VQ], BF16, tag="esc")
                        nc.scalar.activation(esc[:], ps[:, :VQ], AF.Exp,
                                             bias=negm[:, :1],
                                             accum_out=s4[:, gh:gh + 1])
                        lsb = p2scr.tile([128, VQ], BF16, tag="lsb")
                        nc.vector.tensor_copy(lsb[:], ps[:, :VQ])
                        nc.scalar.dma_start(
                            logit_t[128 * g:128 * (g + 1), v0:v0 + VQ], lsb[:])

                # combine the four quarters per group:
                # m = max_q m_q ; s = sum_q s_q * exp(m_q - m)
                m4v = m4[:].rearrange("p (g q) -> p g q", q=4)
                s4v = s4[:].rearrange("p (g q) -> p g q", q=4)
                t01 = p2keep.tile([128, ngrp], F32, tag="t01")
                t23 = p2keep.tile([128, ngrp], F32, tag="t23")
                nc.vector.tensor_max(t01[:], m4v[:, :, 0], m4v[:, :, 1])
                nc.vector.tensor_max(t23[:], m4v[:, :, 2], m4v[:, :, 3])
                nc.vector.tensor_max(m_all[:], t01[:], t23[:])
                acc = p2keep.tile([128, ngrp], F32, tag="sacc")
                dq = p2keep.tile([128, ngrp], F32, tag="dq")
                for q in range(4):
                    nc.vector.tensor_sub(dq[:], m4v[:, :, q], m_all[:])
                    nc.scalar.activation(dq[:], dq[:], AF.Exp)
                    nc.vector.tensor_mul(dq[:], dq[:], s4v[:, :, q])
                    if q == 0:
                        nc.vector.tensor_copy(acc[:], dq[:])
                    else:
                        nc.vector.tensor_add(acc[:], acc[:], dq[:])
                nc.vector.tensor_copy(s_all[:], acc[:])

                # global normalizer: 2 AllReduces over (128, ngrp)
                mloc_d = agp.tile([128, ngrp], F32, tag="mloc")
                mglob_d = agp.tile([128, ngrp], F32, tag="mglob",
                                   addr_space="Shared")
                nc.sync.dma_start(mloc_d[:], m_all[:])
                nc.gpsimd.collective_compute(
                    "AllReduce", mybir.AluOpType.max, replica_groups=RG,
                    ins=[mloc_d[:].opt()], outs=[mglob_d[:].opt()])
                mg_s = p2keep.tile([128, ngrp], F32, tag="mgs")
                nc.sync.dma_start(mg_s[:], mglob_d[:])
                dm = p2keep.tile([128, ngrp], F32, tag="dm")
                nc.vector.tensor_sub(dm[:], m_all[:], mg_s[:])
                edm = p2keep.tile([128, ngrp], F32, tag="edm")
                nc.scalar.activation(edm[:], dm[:], AF.Exp)
                sp = p2keep.tile([128, ngrp], F32, tag="sp")
                nc.vector.tensor_mul(sp[:], s_all[:], edm[:])
                sloc_d = agp.tile([128, ngrp], F32, tag="sloc")
                sglob_d = agp.tile([128, ngrp], F32, tag="sglob",
                                   addr_space="Shared")
                nc.sync.dma_start(sloc_d[:], sp[:])
                nc.gpsimd.collective_compute(
                    "AllReduce", mybir.AluOpType.add, replica_groups=RG,
                    ins=[sloc_d[:].opt()], outs=[sglob_d[:].opt()])
                sg_s = p2keep.tile([128, ngrp], F32, tag="sgs")
                nc.sync.dma_start(sg_s[:], sglob_d[:])
                lns = p2keep.tile([128, ngrp], F32, tag="lns")
                nc.scalar.activation(lns[:], sg_s[:], AF.Ln)
                nc.vector.tensor_add(logz[:], mg_s[:], lns[:])
                nc.sync.dma_start(logz_t[:], logz[:])

    nc.compile()
    return nc


_NC_CACHE = {}


def _get_nc(ts):
    if ts not in _NC_CACHE:
        _NC_CACHE[ts] = build_nc(ts)
    return _NC_CACHE[ts]


def run_device(inputs, ts=TS_FULL, **run_kwargs):
    in_maps, ntok_pad = prep_inputs(inputs, ts)
    nc = _get_nc(ts)
    res = bass_utils.run_bass_kernel_spmd(nc, in_maps,
                                          core_ids=list(range(NC)), **run_kwargs)
    ntok = ts * B
    ngrp = ntok_pad // 128
    logz = np.asarray(res.results[0]["logz"], np.float32)
    logz_flat = logz.T.reshape(-1)[:ntok]          # token-major normalizer
    logp = np.empty((ntok, V), np.float32)
    for c in range(NC):
        logp[:, VS * c:VS * (c + 1)] = np.asarray(
            res.results[c]["logits"][:ntok], np.float32)
    logp -= logz_flat[:, None]
    out = np.zeros((B, T, V), np.float32)
    out[:, 0, SOS_ID] = 1.0
    out[:, 1:1 + ts, :] = logp.reshape(ts, B, V).transpose(1, 0, 2)
    return out, res


def kernel(**inputs) -> np.ndarray:
    out, _ = run_device(inputs, TS_FULL)
    return out
